# revision 13
# baseline (speedup 1.0000x reference)
"""Trainium2 Bass kernel for nn_CountingAbstraction (B=4, N=D=2048), v2.

Math (per example):
    cn   = l2_normalize(data, axis=-1)
    sim  = relu(cn @ cn.T)                      # [N, N], symmetric
    v    = posenc @ sim                         # [N, N]
    csum = sim.sum(-1)                          # [N]
    counter = softplus(concat([csum, v], -1) @ W_exp + b_exp)
    out  = concat([data, counter], -1) @ W_merge

v2 structural changes vs v1:
1. Symmetric gram: only the upper triangle of S0 = dataT.T @ dataT is
   computed by matmul (278K PE cycles instead of 524K); the lower
   triangle of R = relu(S0) is filled by PE transposes of the upper
   blocks (~15K cycles), exploiting relu's monotonicity (R symmetric).
2. posenc has numerical rank ~470 (sharp cliff: sigma_462=21 ->
   sigma_470=9e-4), so P = Us @ V with rank-510 factors (randomized SVD
   on host, deterministic seed; bf16 factors give ~1.5e-3 end-to-end
   error). The old chain  v = (P Dn R) Dn ; pre = v @ W_exp[1:]  (8.6+8.6
   GMAC) becomes   C = (V Dn R)            [512, N]   2.15 GMAC
                   D = (C Dn) @ W1         [512, D]   2.15 GMAC
                   pre_v = Us_half @ D     [NB, D]    1.07 GMAC
   i.e. the W_exp contraction shrinks from N rows to 512 rows. Rank
   slots 480/481 (partition base 96 of rho-tile 3) carry the
   csum*W_exp[0] rank-2 term: vs col 481 is hosted as 1.0 so the
   on-device inv_k scale turns it into inv_k, making Cm[:,:,481] the
   csum column; Dm rows 480/481 are overwritten with w0h (host-masked
   by own half h, keeping the device program h-independent) and us rows
   480/481 with the two csum halves (DMA'd via a DRAM bounce).
3. Layouts are chosen so every stage's output feeds the next stage with
   the contraction dim on partitions - zero transposes outside the gram.

Per-core PE budget (cost-model cycles @2.4GHz): gram 278K + transp 15K
+ A 131K + B 131K + C 82K + merge 524K ~= 1.16M (~484us) vs v1 1.59M.

Sharding: unchanged from v1 - core c = example c//2, output-row half
c%2; all matmul operands bf16 with fp32 PSUM accumulation.
"""

import contextlib

import numpy as np
from ml_dtypes import bfloat16

import concourse.bass as bass
import concourse.mybir as mybir
import concourse.tile as tile
from concourse.vector_clock import ScopedClock
from concourse.bass_utils import run_bass_kernel_spmd

P = 128
FREE = 512  # max matmul moving free dim (one PSUM bank of fp32)
RNK = 512   # numerical rank of the posenc matrix
EPS = 1e-12
F32 = mybir.dt.float32
BF16 = mybir.dt.bfloat16


class SplitDrainTileContext(tile.TileContext):
    """Walrus rejects >1 sync-wait on the kernel-tail Drain; split the waits
    across a chain of single-wait drains on the sync engine."""

    MAX_WAITS = 1

    def _drain_and_barrier(self, tick_clock, wait_clock):
        drain_inst = self.nc.sync.drain()
        wait_clock.add_sem_waits(
            drain_inst.ins, ScopedClock({None: tick_clock.global_clock})
        )
        si = drain_inst.ins.sync_info
        if si is not None and len(si.on_wait) > self.MAX_WAITS:
            waits = list(si.on_wait)
            drain_inst.ins.sync_info = mybir.SyncInfo(
                on_wait=waits[: self.MAX_WAITS], on_update=list(si.on_update)
            )
            for i in range(self.MAX_WAITS, len(waits), self.MAX_WAITS):
                extra = self.nc.sync.drain()
                extra.ins.sync_info = mybir.SyncInfo(
                    on_wait=waits[i : i + self.MAX_WAITS], on_update=[]
                )

        self.nc.all_engine_barrier()
        assert self.sems is not None
        popped = self.nc._tile_sem_poison_stack.pop()
        assert popped is self._sem_poison
        self.nc.clear_and_free_semaphores(list(self.sems.allocated().values()))
        self.nc.all_engine_barrier()


def _free_chunks(total, start=0):
    return [(s, min(FREE, total - s)) for s in range(start, total, FREE)]


def _split_multi_waits(nc, max_waits=1):
    """This walrus build rejects instructions carrying more than one sync
    wait. Hoist extra waits into standalone InstEventSemaphore instructions
    inserted just before the instruction in its engine's stream."""
    n_new = 0
    for fn in nc.m.functions:
        for blk in fn.blocks:
            new_insts = []
            for inst in blk.instructions:
                si = inst.sync_info
                if si is not None and len(si.on_wait) > max_waits:
                    waits = list(si.on_wait)
                    for w in waits[max_waits:]:
                        es = mybir.InstEventSemaphore(
                            name=f"I-hoistw-{n_new}", ins=[], outs=[]
                        )
                        es.engine = inst.engine
                        es.sync_info = mybir.SyncInfo(on_wait=[w], on_update=[])
                        new_insts.append(es)
                        n_new += 1
                    inst.sync_info = mybir.SyncInfo(
                        on_wait=waits[:max_waits], on_update=list(si.on_update)
                    )
                new_insts.append(inst)
            blk.instructions = new_insts
    return n_new


def build_program(N, D, NB, repeat=1, bodies=1, no_dma=False):
    """Emit the SPMD per-core program. N == D, NB = N // 2 (row half)."""
    assert N == D and NB * 2 == N
    NT = N // P          # contraction / row tiles
    RT = RNK // P        # posenc-rank tiles
    CT = 2 * NT          # merge contraction tiles
    ET = NT              # output feature tiles

    nc = bass.Bass("TRN2", num_devices=8)

    dataT = nc.dram_tensor("dataT", [N, N], BF16, kind="ExternalInput")
    dataTnb = nc.dram_tensor("dataTnb", [N, NB], BF16, kind="ExternalInput")
    vsd = nc.dram_tensor("vsd", [N, RNK], BF16, kind="ExternalInput")
    usd = nc.dram_tensor("usd", [RNK, NB], BF16, kind="ExternalInput")
    w1d = nc.dram_tensor("w1d", [N, D], BF16, kind="ExternalInput")
    w0h = nc.dram_tensor("w0h", [P, D], BF16, kind="ExternalInput")
    wm = nc.dram_tensor("wm", [ET, P, CT, P], BF16, kind="ExternalInput")
    bexp = nc.dram_tensor("bexp", [D], F32, kind="ExternalInput")
    identb = nc.dram_tensor("identb", [P, P], BF16, kind="ExternalInput")
    outT = nc.dram_tensor("outT", [D, NB], F32, kind="ExternalOutput")

    dataT_t = dataT.rearrange("(o p) f -> p o f", p=P)
    dataTnb_t = dataTnb.rearrange("(o p) f -> p o f", p=P)
    vsd_t = vsd.rearrange("(o p) f -> p o f", p=P)
    usd_t = usd.rearrange("(o p) f -> p o f", p=P)
    w1d_t = w1d.rearrange("(o p) f -> p o f", p=P)
    bexp_t = bexp.rearrange("(o p) -> p o", p=P)
    outT_t = outT.rearrange("(o p) f -> p o f", p=P)

    with SplitDrainTileContext(nc) as tc:
        with (
            tc.tile_pool(name="big", bufs=1) as big,
            tc.tile_pool(name="mid", bufs=1) as mid,
            tc.tile_pool(name="small", bufs=1) as small,
            tc.tile_pool(name="evict", bufs=2) as evict,
            tc.tile_pool(name="ps", bufs=2, space="PSUM") as ps,
            tc.tile_pool(name="dram", bufs=1, space="DRAM") as dram,
        ):
            def emit_body():
                # ---- resident tensors ---------------------------------
                # tag "huge": two 64KB slots; lifetime chains
                #   slot A: dT (gram) -> w1sb (stage B)
                #   slot B: R (gram..stage A) -> dnbus (stage C/5; the
                #           dnb+us combo tile rides in the 64KB slot, so
                #           the us DMA overlaps stage B instead of gating
                #           stage C)
                # tag "mid": two 16KB slots; chain
                #   slot A: vs (stage A) -> Dm (stage B..C) -> wm stream
                #   slot B: Cm (stage A..B) -> wmt0 prefetch -> wm stream
                dT = big.tile([P, NT, N], BF16, tag="huge", bufs=2)
                R = big.tile([P, NT, N], BF16, tag="huge", bufs=2)
                counterT = big.tile([P, NT, NB], BF16, tag="ct", bufs=1)

                vs = mid.tile([P, NT, RNK], BF16, tag="mid", bufs=2)

                bexp_sb = small.tile([P, NT], F32, tag="bexp")
                nrm2 = small.tile([P, NT], F32, tag="nrm2")
                nrm = small.tile([P, NT], F32, tag="nrm")
                inv = small.tile([P, NT], F32, tag="inv")
                csum_col_bf = small.tile([P, NT], BF16, tag="csum_col_bf")
                w0h_sb = small.tile([P, D], BF16, tag="w0h")
                identb_sb = small.tile([P, P], BF16, tag="identb")
                diag_tmp = small.tile([P, P], F32, tag="diag_tmp")

                dram_csum = dram.tile([N], BF16)

                # ---- input DMAs -----------------------------------
                # dT chunks are emitted FIRST so round-robin queue
                # assignment gives each of the 8 chunks its own DMA queue
                # (the refill at every For_i iteration start is paced by
                # these); the small inputs follow behind.
                lc = max(1, NT // 8)
                if not no_dma:
                    for o in range(0, NT, lc):
                        nc.sync.dma_start(
                            dT[:, o : o + lc, :], dataT_t[:, o : o + lc, :]
                        )
                    nc.sync.dma_start(vs[:], vsd_t)
                nc.sync.dma_start(identb_sb[:], identb[:])
                nc.sync.dma_start(bexp_sb[:], bexp_t)
                nc.sync.dma_start(w0h_sb[:], w0h[:])

                # ---- stage 1: symmetric gram --------------------------
                # row-tile it computes cols [it*P, N) (upper triangle);
                # lower blocks are PE transposes of earlier rows' upper.
                def emit_transposes(row):
                    rlo = row * P
                    psT = ps.tile([P, 4096], BF16, tag="ps", name="psT")
                    for j in range(row):
                        nc.tensor.transpose(
                            psT[:, j * P : (j + 1) * P],
                            R[:, j, rlo : rlo + P],
                            identb_sb[:],
                        )
                    nc.scalar.activation(
                        R[:, row, 0:rlo],
                        psT[:, 0:rlo],
                        mybir.ActivationFunctionType.Copy,
                    )

                for it in range(NT):
                    lo = it * P
                    L = N - lo
                    psg = ps.tile([P, 2048], F32, tag="ps")
                    for dt_ in range(NT):
                        lhsT = dT[:, dt_, lo : lo + P]
                        for (s, w) in _free_chunks(N, start=lo):
                            nc.tensor.matmul(
                                psg[:, s - lo : s - lo + w],
                                lhsT,
                                dT[:, dt_, s : s + w],
                                start=(dt_ == 0),
                                stop=(dt_ == NT - 1),
                            )
                    if it > 1:
                        emit_transposes(it - 1)
                    # diagonal block (local cols 0:P) -> squared norms
                    nc.vector.tensor_tensor(
                        diag_tmp[:],
                        psg[:, 0:P],
                        identb_sb[:],
                        mybir.AluOpType.mult,
                    )
                    nc.vector.reduce_sum(
                        nrm2[:, it : it + 1],
                        diag_tmp[:],
                        axis=mybir.AxisListType.X,
                    )
                    # inv and the vs scale for this k-tile, computed here so
                    # stage A starts with zero latency after the gram
                    nc.vector.tensor_scalar_max(
                        nrm2[:, it : it + 1], nrm2[:, it : it + 1], EPS
                    )
                    nc.scalar.sqrt(nrm[:, it : it + 1], nrm2[:, it : it + 1])
                    nc.vector.reciprocal(inv[:, it : it + 1], nrm[:, it : it + 1])
                    nc.vector.tensor_scalar_mul(
                        vs[:, it, :], vs[:, it, :], inv[:, it : it + 1]
                    )
                    nc.scalar.activation(
                        R[:, it, lo:N],
                        psg[:, 0:L],
                        mybir.ActivationFunctionType.Relu,
                    )

                emit_transposes(NT - 1)

                # w1 load into dT's slot (dT dead after the gram)
                w1sb = big.tile([P, NT, D], BF16, tag="huge", bufs=2)
                if not no_dma:
                    for o in range(0, NT, lc):
                        nc.sync.dma_start(
                            w1sb[:, o : o + lc, :], w1d_t[:, o : o + lc, :]
                        )

                # ---- stage A: Cm[m, rho] = inv_m * sum_k R[k,m] vs[k,rho]
                # vs col 511 is hosted as 1.0, so the inv_k scale turns it
                # into inv_k and Cm[:,:,511] = inv_m sum_k R[k,m] inv_k =
                # csum[m]; us row 511 is 0 so it drops out of stage C.
                Cm = mid.tile([P, NT, RNK], BF16, tag="mid", bufs=2)
                for mt in range(NT):
                    psA = ps.tile([P, 2048], F32, tag="ps")
                    for kt in range(NT):
                        lhsT = R[:, kt, mt * P : (mt + 1) * P]
                        nc.tensor.matmul(
                            psA[:, 0:RNK],
                            lhsT,
                            vs[:, kt, :],
                            start=(kt == 0),
                            stop=(kt == NT - 1),
                        )
                    nc.scalar.activation(
                        Cm[:, mt, :],
                        psA[:, 0:RNK],
                        mybir.ActivationFunctionType.Copy,
                        scale=inv[:, mt : mt + 1],
                    )

                # csum column -> DRAM (row-major by m); read back later
                # directly into us partitions 126:128 (rho rows 510/511)
                nc.vector.tensor_copy(
                    csum_col_bf[:],
                    Cm[:, :, 481],
                )
                nc.sync.dma_start(
                    dram_csum.rearrange("(o p) -> p o", p=P), csum_col_bf[:]
                )

                # dnb + us share one tile in R's slot (R dead after stage A):
                # us loads during stage B instead of gating stage C's start.
                dnbus = big.tile([P, NT + RT, NB], BF16, tag="huge", bufs=2)
                dnb = dnbus[:, 0:NT, :]
                us = dnbus[:, NT : NT + RT, :]
                if not no_dma:
                    for o in range(0, NT, lc):
                        nc.sync.dma_start(
                            dnbus[:, o : o + lc, :], dataTnb_t[:, o : o + lc, :]
                        )
                    for rt in range(RT):
                        nc.sync.dma_start(
                            dnbus[:, NT + rt : NT + rt + 1, :],
                            usd_t[:, rt : rt + 1, :],
                        )
                nc.sync.dma_start(
                    dnbus[96:98, NT + RT - 1, :],
                    dram_csum.rearrange("(h n) -> h n", h=2),
                )

                # ---- stage B: Dm[rho, d] = sum_m Cm[m, rho] w1[m, d]
                Dm = mid.tile([P, RT, D], BF16, tag="mid", bufs=2)
                for rt in range(RT):
                    psB = ps.tile([P, 2048], F32, tag="ps")
                    for mt in range(NT):
                        lhsT = Cm[:, mt, rt * P : (rt + 1) * P]
                        for (s, w) in _free_chunks(D):
                            nc.tensor.matmul(
                                psB[:, s : s + w],
                                lhsT,
                                w1sb[:, mt, s : s + w],
                                start=(mt == 0),
                                stop=(mt == NT - 1),
                            )
                    if rt == RT - 1:
                        nc.scalar.activation(
                            Dm[:, rt, 0 : D // 2],
                            psB[:, 0 : D // 2],
                            mybir.ActivationFunctionType.Copy,
                        )
                        nc.scalar.activation(
                            Dm[:, rt, D // 2 : D],
                            psB[:, D // 2 : D],
                            mybir.ActivationFunctionType.Copy,
                        )
                    else:
                        nc.scalar.activation(
                            Dm[:, rt, :],
                            psB[:, 0:D],
                            mybir.ActivationFunctionType.Copy,
                        )
                nc.vector.tensor_copy(
                    Dm[96:98, RT - 1, :], w0h_sb[96:98, :]
                )

                # prefetch the first wm tile into Cm's freed mid slot so its
                # DMA overlaps stage C instead of stalling stage 5's start
                wmt0 = mid.tile([P, NT, P], BF16, tag="mid", bufs=2)
                if not no_dma:
                    nc.sync.dma_start(wmt0[:], wm[0, :, 0:NT, :])

                # ---- stage C: counterT[d, n] = softplus(
                #   sum_rho Dm[rho, d] us[rho, n] + bexp[d])
                # (rho rows 510/511 carry the w0 x csum term)
                for dt_ in range(NT):
                    psC = ps.tile([P, 2048], F32, tag="ps")
                    dsl = slice(dt_ * P, (dt_ + 1) * P)
                    for rt in range(RT):
                        lhsT = Dm[:, rt, dsl]
                        for (s, w) in _free_chunks(NB):
                            nc.tensor.matmul(
                                psC[:, s : s + w],
                                lhsT,
                                us[:, rt, s : s + w],
                                start=(rt == 0),
                                stop=(rt == RT - 1),
                            )
                    # softplus(x + b) as ln(exp(x + b) + 1)
                    spt = evict.tile([P, NB], F32, tag="ev")
                    nc.scalar.activation(
                        spt[:],
                        psC[:, 0:NB],
                        mybir.ActivationFunctionType.Exp,
                        bias=bexp_sb[:, dt_ : dt_ + 1],
                    )
                    nc.scalar.activation(
                        counterT[:, dt_, :],
                        spt[:],
                        mybir.ActivationFunctionType.Ln,
                        bias=1.0,
                    )

                # ---- stage 5: outT[e, n] = sum_c wm[c, e]*[dnb; counterT][c, n]
                for et in range(ET):
                    psO = ps.tile([P, 2048], F32, tag="ps")
                    for h2 in range(2):
                        if et == 0 and h2 == 0:
                            wmt = wmt0
                        else:
                            wmt = mid.tile([P, NT, P], BF16, tag="mid", bufs=2)
                            if not no_dma:
                                nc.sync.dma_start(
                                    wmt[:], wm[et, :, h2 * NT : (h2 + 1) * NT, :]
                                )
                        for ci in range(NT):
                            ct = h2 * NT + ci
                            lhsT = wmt[:, ci, :]
                            rhs_tile = (
                                dnb[:, ct, :]
                                if ct < NT
                                else counterT[:, ct - NT, :]
                            )
                            for (s, w) in _free_chunks(NB):
                                nc.tensor.matmul(
                                    psO[:, s : s + w],
                                    lhsT,
                                    rhs_tile[:, s : s + w],
                                    start=(ct == 0),
                                    stop=(ct == CT - 1),
                                )
                    osb = evict.tile([P, NB], F32, tag="ev")
                    nc.vector.tensor_copy(osb[:], psO[:, 0:NB])
                    nc.sync.dma_start(outT_t[:, et, :], osb[:])

            with tc.For_i(0, repeat) if repeat > 1 else contextlib.nullcontext():
                for _ in range(bodies):
                    emit_body()

    _split_multi_waits(nc)
    return nc


# ---------------------------------------------------------------------------
# host side
# ---------------------------------------------------------------------------

def get_posenc(n, d):
    pos = np.arange(n)[:, None].astype(np.float32)
    i = np.arange(d)[None, :]
    angle_rates = 1.0 / np.power(
        10000.0, (2 * (i // 2)).astype(np.float32) / np.float32(d)
    )
    angles = pos * angle_rates
    pe = np.zeros((n, d), dtype=np.float32)
    pe[:, 0::2] = np.sin(angles[:, 0::2])
    pe[:, 1::2] = np.cos(angles[:, 1::2])
    return pe


_posenc_cache = {}


def posenc_factors(n, d, r=RNK - 2):
    """P ~= Us @ V exactly to ~1e-6 relative: the sinusoidal posenc has a
    sharp numerical-rank cliff at ~470. Randomized SVD (deterministic seed,
    one power iteration) captures the top-r subspace to machine precision."""
    if (n, d, r) in _posenc_cache:
        return _posenc_cache[(n, d, r)]
    Pm = get_posenc(n, d).astype(np.float64)
    rng = np.random.default_rng(12345)
    G = rng.standard_normal((d, r + 32))
    Y = Pm @ G
    Y = Pm @ (Pm.T @ Y)
    Q, _ = np.linalg.qr(Y)
    Bm = Q.T @ Pm
    Ub, s, Vt = np.linalg.svd(Bm, full_matrices=False)
    Us = (Q @ Ub[:, :r]) * s[:r][None, :]
    V = Vt[:r, :]
    out = (Us.astype(np.float32), V.astype(np.float32))
    _posenc_cache[(n, d, r)] = out
    return out


def _host_prep(data, W_exp, b_exp, W_merge):
    """Layout-only host prep (plus the constant posenc factorization);
    returns per-core input maps."""
    B, N, D = data.shape
    NB = N // 2
    NT = N // P
    CT = 2 * NT
    ET = NT

    # rank RNK-2 posenc; rho slots 480/481 (partition base 96 of tile 3)
    # carry the w0*csum rank-2 term: vs col 481 is 1.0 so the on-device
    # inv_k scale turns it into inv_k and Cm[:,:,481] becomes csum.
    Us, V = posenc_factors(N, D)
    pe_slots = np.r_[0:480, 482:RNK]
    vs = np.zeros((N, RNK), dtype=bfloat16)
    vs[:, pe_slots] = V.T.astype(bfloat16)
    vs[:, 481] = 1.0
    us_h = []
    for h in range(2):
        u = np.zeros((RNK, NB), dtype=bfloat16)
        u[pe_slots] = Us[h * NB : (h + 1) * NB, :].T.astype(bfloat16)
        us_h.append(u)

    w1 = np.ascontiguousarray(W_exp[1:].astype(bfloat16))    # [N, D]
    w0h_v = []
    for h in range(2):
        w0h = np.zeros((P, D), dtype=bfloat16)
        w0h[96 + h] = W_exp[0].astype(bfloat16)
        w0h_v.append(w0h)

    wm_s = np.ascontiguousarray(
        W_merge.astype(bfloat16).reshape(CT, P, ET, P).transpose(2, 1, 0, 3)
    )  # [et, p, ct, f]

    bexp_f = np.ascontiguousarray(b_exp.astype(np.float32))

    dataT_b = [np.ascontiguousarray(data[b].T.astype(bfloat16)) for b in range(B)]

    identb = np.eye(P, dtype=np.float32).astype(bfloat16)

    in_maps = []
    for c in range(2 * B):
        b, h = c // 2, c % 2
        nb = slice(h * NB, (h + 1) * NB)
        m = {
            "dataT": dataT_b[b],
            "dataTnb": np.ascontiguousarray(dataT_b[b][:, nb]),
            "vsd": vs,
            "usd": us_h[h],
            "w1d": w1,
            "w0h": w0h_v[h],
            "wm": wm_s,
            "bexp": bexp_f,
            "identb": identb,
        }
        in_maps.append(m)
    return in_maps


_program_cache = {}


def _get_program(N, D, NB, repeat=1, bodies=1, no_dma=False):
    key = (N, D, NB, repeat, bodies, no_dma)
    if key not in _program_cache:
        _program_cache[key] = build_program(
            N, D, NB, repeat=repeat, bodies=bodies, no_dma=no_dma
        )
    return _program_cache[key]


def kernel(data, W_exp, b_exp, W_merge):
    data = np.asarray(data)
    W_exp = np.asarray(W_exp)
    b_exp = np.asarray(b_exp)
    W_merge = np.asarray(W_merge)
    B, N, D = data.shape
    NB = N // 2

    nc = _get_program(N, D, NB)
    in_maps = _host_prep(data, W_exp, b_exp, W_merge)
    core_ids = list(range(2 * B))
    res = run_bass_kernel_spmd(nc, in_maps, core_ids)

    out = np.empty((B, N, D), dtype=np.float32)
    for c in core_ids:
        b, h = c // 2, c % 2
        out[b, h * NB : (h + 1) * NB, :] = res.results[c]["outT"].T
    return out


# revision 14
# speedup vs baseline: 1.0025x; 1.0025x over previous
"""Trainium2 Bass kernel for nn_CountingAbstraction (B=4, N=D=2048), v2.

Math (per example):
    cn   = l2_normalize(data, axis=-1)
    sim  = relu(cn @ cn.T)                      # [N, N], symmetric
    v    = posenc @ sim                         # [N, N]
    csum = sim.sum(-1)                          # [N]
    counter = softplus(concat([csum, v], -1) @ W_exp + b_exp)
    out  = concat([data, counter], -1) @ W_merge

v2 structural changes vs v1:
1. Symmetric gram: only the upper triangle of S0 = dataT.T @ dataT is
   computed by matmul (278K PE cycles instead of 524K); the lower
   triangle of R = relu(S0) is filled by PE transposes of the upper
   blocks (~15K cycles), exploiting relu's monotonicity (R symmetric).
2. posenc has numerical rank ~470 (sharp cliff: sigma_462=21 ->
   sigma_470=9e-4), so P = Us @ V with rank-510 factors (randomized SVD
   on host, deterministic seed; bf16 factors give ~1.5e-3 end-to-end
   error). The old chain  v = (P Dn R) Dn ; pre = v @ W_exp[1:]  (8.6+8.6
   GMAC) becomes   C = (V Dn R)            [512, N]   2.15 GMAC
                   D = (C Dn) @ W1         [512, D]   2.15 GMAC
                   pre_v = Us_half @ D     [NB, D]    1.07 GMAC
   i.e. the W_exp contraction shrinks from N rows to 512 rows. Rank
   slots 480/481 (partition base 96 of rho-tile 3) carry the
   csum*W_exp[0] rank-2 term: vs col 481 is hosted as 1.0 so the
   on-device inv_k scale turns it into inv_k, making Cm[:,:,481] the
   csum column; Dm rows 480/481 are overwritten with w0h (host-masked
   by own half h, keeping the device program h-independent) and us rows
   480/481 with the two csum halves (DMA'd via a DRAM bounce).
3. Layouts are chosen so every stage's output feeds the next stage with
   the contraction dim on partitions - zero transposes outside the gram.

Per-core PE budget (cost-model cycles @2.4GHz): gram 278K + transp 15K
+ A 131K + B 131K + C 82K + merge 524K ~= 1.16M (~484us) vs v1 1.59M.

Sharding: unchanged from v1 - core c = example c//2, output-row half
c%2; all matmul operands bf16 with fp32 PSUM accumulation.
"""

import contextlib

import numpy as np
from ml_dtypes import bfloat16

import concourse.bass as bass
import concourse.mybir as mybir
import concourse.tile as tile
from concourse.vector_clock import ScopedClock
from concourse.bass_utils import run_bass_kernel_spmd

P = 128
FREE = 512  # max matmul moving free dim (one PSUM bank of fp32)
RNK = 512   # numerical rank of the posenc matrix
EPS = 1e-12
F32 = mybir.dt.float32
BF16 = mybir.dt.bfloat16


class SplitDrainTileContext(tile.TileContext):
    """Walrus rejects >1 sync-wait on the kernel-tail Drain; split the waits
    across a chain of single-wait drains on the sync engine."""

    MAX_WAITS = 1

    def _drain_and_barrier(self, tick_clock, wait_clock):
        drain_inst = self.nc.sync.drain()
        wait_clock.add_sem_waits(
            drain_inst.ins, ScopedClock({None: tick_clock.global_clock})
        )
        si = drain_inst.ins.sync_info
        if si is not None and len(si.on_wait) > self.MAX_WAITS:
            waits = list(si.on_wait)
            drain_inst.ins.sync_info = mybir.SyncInfo(
                on_wait=waits[: self.MAX_WAITS], on_update=list(si.on_update)
            )
            for i in range(self.MAX_WAITS, len(waits), self.MAX_WAITS):
                extra = self.nc.sync.drain()
                extra.ins.sync_info = mybir.SyncInfo(
                    on_wait=waits[i : i + self.MAX_WAITS], on_update=[]
                )

        self.nc.all_engine_barrier()
        assert self.sems is not None
        popped = self.nc._tile_sem_poison_stack.pop()
        assert popped is self._sem_poison
        self.nc.clear_and_free_semaphores(list(self.sems.allocated().values()))
        self.nc.all_engine_barrier()


def _free_chunks(total, start=0):
    return [(s, min(FREE, total - s)) for s in range(start, total, FREE)]


def _split_multi_waits(nc, max_waits=1):
    """This walrus build rejects instructions carrying more than one sync
    wait. Hoist extra waits into standalone InstEventSemaphore instructions
    inserted just before the instruction in its engine's stream."""
    n_new = 0
    for fn in nc.m.functions:
        for blk in fn.blocks:
            new_insts = []
            for inst in blk.instructions:
                si = inst.sync_info
                if si is not None and len(si.on_wait) > max_waits:
                    waits = list(si.on_wait)
                    for w in waits[max_waits:]:
                        es = mybir.InstEventSemaphore(
                            name=f"I-hoistw-{n_new}", ins=[], outs=[]
                        )
                        es.engine = inst.engine
                        es.sync_info = mybir.SyncInfo(on_wait=[w], on_update=[])
                        new_insts.append(es)
                        n_new += 1
                    inst.sync_info = mybir.SyncInfo(
                        on_wait=waits[:max_waits], on_update=list(si.on_update)
                    )
                new_insts.append(inst)
            blk.instructions = new_insts
    return n_new


def build_program(N, D, NB, repeat=1, bodies=1, no_dma=False):
    """Emit the SPMD per-core program. N == D, NB = N // 2 (row half)."""
    assert N == D and NB * 2 == N
    NT = N // P          # contraction / row tiles
    RT = RNK // P        # posenc-rank tiles
    CT = 2 * NT          # merge contraction tiles
    ET = NT              # output feature tiles

    nc = bass.Bass("TRN2", num_devices=8)

    dataT = nc.dram_tensor("dataT", [N, N], BF16, kind="ExternalInput")
    dataTnb = nc.dram_tensor("dataTnb", [N, NB], BF16, kind="ExternalInput")
    vsd = nc.dram_tensor("vsd", [N, RNK], BF16, kind="ExternalInput")
    usd = nc.dram_tensor("usd", [RNK, NB], BF16, kind="ExternalInput")
    w1d = nc.dram_tensor("w1d", [N, D], BF16, kind="ExternalInput")
    w0h = nc.dram_tensor("w0h", [P, D], BF16, kind="ExternalInput")
    wm = nc.dram_tensor("wm", [ET, P, CT, P], BF16, kind="ExternalInput")
    bexp = nc.dram_tensor("bexp", [D], F32, kind="ExternalInput")
    identb = nc.dram_tensor("identb", [P, P], BF16, kind="ExternalInput")
    outT = nc.dram_tensor("outT", [D, NB], F32, kind="ExternalOutput")

    dataT_t = dataT.rearrange("(o p) f -> p o f", p=P)
    dataTnb_t = dataTnb.rearrange("(o p) f -> p o f", p=P)
    vsd_t = vsd.rearrange("(o p) f -> p o f", p=P)
    usd_t = usd.rearrange("(o p) f -> p o f", p=P)
    w1d_t = w1d.rearrange("(o p) f -> p o f", p=P)
    bexp_t = bexp.rearrange("(o p) -> p o", p=P)
    outT_t = outT.rearrange("(o p) f -> p o f", p=P)

    with SplitDrainTileContext(nc) as tc:
        with (
            tc.tile_pool(name="big", bufs=1) as big,
            tc.tile_pool(name="mid", bufs=1) as mid,
            tc.tile_pool(name="small", bufs=1) as small,
            tc.tile_pool(name="evict", bufs=2) as evict,
            tc.tile_pool(name="ps", bufs=2, space="PSUM") as ps,
            tc.tile_pool(name="dram", bufs=1, space="DRAM") as dram,
        ):
            def emit_body():
                # ---- resident tensors ---------------------------------
                # tag "huge": two 64KB slots; lifetime chains
                #   slot A: dT (gram) -> w1sb (stage B)
                #   slot B: R (gram..stage A) -> dnbus (stage C/5; the
                #           dnb+us combo tile rides in the 64KB slot, so
                #           the us DMA overlaps stage B instead of gating
                #           stage C)
                # tag "mid": two 16KB slots; chain
                #   slot A: vs (stage A) -> Dm (stage B..C) -> wm stream
                #   slot B: Cm (stage A..B) -> wmt0 prefetch -> wm stream
                dT = big.tile([P, NT, N], BF16, tag="huge", bufs=2)
                R = big.tile([P, NT, N], BF16, tag="huge", bufs=2)
                counterT = big.tile([P, NT, NB], BF16, tag="ct", bufs=1)

                vs = mid.tile([P, NT, RNK], BF16, tag="mid", bufs=2)

                bexp_sb = small.tile([P, NT], F32, tag="bexp")
                nrm2 = small.tile([P, NT], F32, tag="nrm2")
                nrm = small.tile([P, NT], F32, tag="nrm")
                inv = small.tile([P, NT], F32, tag="inv")
                csum_col_bf = small.tile([P, NT], BF16, tag="csum_col_bf")
                w0h_sb = small.tile([P, D], BF16, tag="w0h")
                identb_sb = small.tile([P, P], BF16, tag="identb")
                diag_tmp = small.tile([P, P], F32, tag="diag_tmp")

                dram_csum = dram.tile([N], BF16)

                # ---- input DMAs (chunked across queues) ---------------
                nc.sync.dma_start(identb_sb[:], identb[:])
                nc.sync.dma_start(bexp_sb[:], bexp_t)
                nc.sync.dma_start(w0h_sb[:], w0h[:])
                if not no_dma:
                    nc.sync.dma_start(vs[:], vsd_t)
                lc = max(1, NT // 8)
                if not no_dma:
                    for o in range(0, NT, lc):
                        nc.sync.dma_start(
                            dT[:, o : o + lc, :], dataT_t[:, o : o + lc, :]
                        )

                # ---- stage 1: symmetric gram --------------------------
                # row-tile it computes cols [it*P, N) (upper triangle);
                # lower blocks are PE transposes of earlier rows' upper.
                def emit_transposes(row):
                    rlo = row * P
                    psT = ps.tile([P, 4096], BF16, tag="ps", name="psT")
                    for j in range(row):
                        nc.tensor.transpose(
                            psT[:, j * P : (j + 1) * P],
                            R[:, j, rlo : rlo + P],
                            identb_sb[:],
                        )
                    nc.scalar.activation(
                        R[:, row, 0:rlo],
                        psT[:, 0:rlo],
                        mybir.ActivationFunctionType.Copy,
                    )

                for it in range(NT):
                    lo = it * P
                    L = N - lo
                    psg = ps.tile([P, 2048], F32, tag="ps")
                    for dt_ in range(NT):
                        lhsT = dT[:, dt_, lo : lo + P]
                        for (s, w) in _free_chunks(N, start=lo):
                            nc.tensor.matmul(
                                psg[:, s - lo : s - lo + w],
                                lhsT,
                                dT[:, dt_, s : s + w],
                                start=(dt_ == 0),
                                stop=(dt_ == NT - 1),
                            )
                    if it > 1:
                        emit_transposes(it - 1)
                    # diagonal block (local cols 0:P) -> squared norms
                    nc.vector.tensor_tensor(
                        diag_tmp[:],
                        psg[:, 0:P],
                        identb_sb[:],
                        mybir.AluOpType.mult,
                    )
                    nc.vector.reduce_sum(
                        nrm2[:, it : it + 1],
                        diag_tmp[:],
                        axis=mybir.AxisListType.X,
                    )
                    # inv and the vs scale for this k-tile, computed here so
                    # stage A starts with zero latency after the gram
                    nc.vector.tensor_scalar_max(
                        nrm2[:, it : it + 1], nrm2[:, it : it + 1], EPS
                    )
                    nc.scalar.sqrt(nrm[:, it : it + 1], nrm2[:, it : it + 1])
                    nc.vector.reciprocal(inv[:, it : it + 1], nrm[:, it : it + 1])
                    nc.vector.tensor_scalar_mul(
                        vs[:, it, :], vs[:, it, :], inv[:, it : it + 1]
                    )
                    nc.scalar.activation(
                        R[:, it, lo:N],
                        psg[:, 0:L],
                        mybir.ActivationFunctionType.Relu,
                    )

                emit_transposes(NT - 1)

                # w1 load into dT's slot (dT dead after the gram)
                w1sb = big.tile([P, NT, D], BF16, tag="huge", bufs=2)
                if not no_dma:
                    for o in range(0, NT, lc):
                        nc.sync.dma_start(
                            w1sb[:, o : o + lc, :], w1d_t[:, o : o + lc, :]
                        )

                # ---- stage A: Cm[m, rho] = inv_m * sum_k R[k,m] vs[k,rho]
                # vs col 511 is hosted as 1.0, so the inv_k scale turns it
                # into inv_k and Cm[:,:,511] = inv_m sum_k R[k,m] inv_k =
                # csum[m]; us row 511 is 0 so it drops out of stage C.
                Cm = mid.tile([P, NT, RNK], BF16, tag="mid", bufs=2)
                for mt in range(NT):
                    psA = ps.tile([P, 2048], F32, tag="ps")
                    for kt in range(NT):
                        lhsT = R[:, kt, mt * P : (mt + 1) * P]
                        nc.tensor.matmul(
                            psA[:, 0:RNK],
                            lhsT,
                            vs[:, kt, :],
                            start=(kt == 0),
                            stop=(kt == NT - 1),
                        )
                    nc.scalar.activation(
                        Cm[:, mt, :],
                        psA[:, 0:RNK],
                        mybir.ActivationFunctionType.Copy,
                        scale=inv[:, mt : mt + 1],
                    )

                # csum column -> DRAM (row-major by m); read back later
                # directly into us partitions 126:128 (rho rows 510/511)
                nc.vector.tensor_copy(
                    csum_col_bf[:],
                    Cm[:, :, 481],
                )
                nc.sync.dma_start(
                    dram_csum.rearrange("(o p) -> p o", p=P), csum_col_bf[:]
                )

                # dnb + us share one tile in R's slot (R dead after stage A):
                # us loads during stage B instead of gating stage C's start.
                dnbus = big.tile([P, NT + RT, NB], BF16, tag="huge", bufs=2)
                dnb = dnbus[:, 0:NT, :]
                us = dnbus[:, NT : NT + RT, :]
                if not no_dma:
                    for o in range(0, NT, lc):
                        nc.sync.dma_start(
                            dnbus[:, o : o + lc, :], dataTnb_t[:, o : o + lc, :]
                        )
                    for rt in range(RT):
                        nc.sync.dma_start(
                            dnbus[:, NT + rt : NT + rt + 1, :],
                            usd_t[:, rt : rt + 1, :],
                        )
                nc.sync.dma_start(
                    dnbus[96:98, NT + RT - 1, :],
                    dram_csum.rearrange("(h n) -> h n", h=2),
                )

                # ---- stage B: Dm[rho, d] = sum_m Cm[m, rho] w1[m, d]
                Dm = mid.tile([P, RT, D], BF16, tag="mid", bufs=2)
                for rt in range(RT):
                    psB = ps.tile([P, 2048], F32, tag="ps")
                    for mt in range(NT):
                        lhsT = Cm[:, mt, rt * P : (rt + 1) * P]
                        for (s, w) in _free_chunks(D):
                            nc.tensor.matmul(
                                psB[:, s : s + w],
                                lhsT,
                                w1sb[:, mt, s : s + w],
                                start=(mt == 0),
                                stop=(mt == NT - 1),
                            )
                    if rt == RT - 1:
                        nc.scalar.activation(
                            Dm[:, rt, 0 : D // 2],
                            psB[:, 0 : D // 2],
                            mybir.ActivationFunctionType.Copy,
                        )
                        nc.scalar.activation(
                            Dm[:, rt, D // 2 : D],
                            psB[:, D // 2 : D],
                            mybir.ActivationFunctionType.Copy,
                        )
                    else:
                        nc.scalar.activation(
                            Dm[:, rt, :],
                            psB[:, 0:D],
                            mybir.ActivationFunctionType.Copy,
                        )
                nc.vector.tensor_copy(
                    Dm[96:98, RT - 1, :], w0h_sb[96:98, :]
                )

                # prefetch the first wm tile into Cm's freed mid slot so its
                # DMA overlaps stage C instead of stalling stage 5's start
                wmt0 = mid.tile([P, NT, P], BF16, tag="mid", bufs=2)
                if not no_dma:
                    nc.sync.dma_start(wmt0[:], wm[0, :, 0:NT, :])

                # ---- stage C: counterT[d, n] = softplus(
                #   sum_rho Dm[rho, d] us[rho, n] + bexp[d])
                # (rho rows 510/511 carry the w0 x csum term)
                for dt_ in range(NT):
                    psC = ps.tile([P, 2048], F32, tag="ps")
                    dsl = slice(dt_ * P, (dt_ + 1) * P)
                    for rt in range(RT):
                        lhsT = Dm[:, rt, dsl]
                        for (s, w) in _free_chunks(NB):
                            nc.tensor.matmul(
                                psC[:, s : s + w],
                                lhsT,
                                us[:, rt, s : s + w],
                                start=(rt == 0),
                                stop=(rt == RT - 1),
                            )
                    # softplus(x + b) as ln(exp(x + b) + 1)
                    spt = evict.tile([P, NB], F32, tag="ev")
                    nc.scalar.activation(
                        spt[:],
                        psC[:, 0:NB],
                        mybir.ActivationFunctionType.Exp,
                        bias=bexp_sb[:, dt_ : dt_ + 1],
                    )
                    nc.scalar.activation(
                        counterT[:, dt_, :],
                        spt[:],
                        mybir.ActivationFunctionType.Ln,
                        bias=1.0,
                    )

                # ---- stage 5: outT[e, n] = sum_c wm[c, e]*[dnb; counterT][c, n]
                for et in range(ET):
                    psO = ps.tile([P, 2048], F32, tag="ps")
                    for h2 in range(2):
                        if et == 0 and h2 == 0:
                            wmt = wmt0
                        else:
                            wmt = mid.tile([P, NT, P], BF16, tag="mid", bufs=2)
                            if not no_dma:
                                nc.sync.dma_start(
                                    wmt[:], wm[et, :, h2 * NT : (h2 + 1) * NT, :]
                                )
                        for ci in range(NT):
                            ct = h2 * NT + ci
                            lhsT = wmt[:, ci, :]
                            rhs_tile = (
                                dnb[:, ct, :]
                                if ct < NT
                                else counterT[:, ct - NT, :]
                            )
                            for (s, w) in _free_chunks(NB):
                                nc.tensor.matmul(
                                    psO[:, s : s + w],
                                    lhsT,
                                    rhs_tile[:, s : s + w],
                                    start=(ct == 0),
                                    stop=(ct == CT - 1),
                                )
                    osb = evict.tile([P, NB], F32, tag="ev")
                    nc.vector.tensor_copy(osb[:], psO[:, 0:NB])
                    nc.sync.dma_start(outT_t[:, et, :], osb[:])

            with tc.For_i(0, repeat) if repeat > 1 else contextlib.nullcontext():
                for _ in range(bodies):
                    emit_body()

    _split_multi_waits(nc)
    return nc


# ---------------------------------------------------------------------------
# host side
# ---------------------------------------------------------------------------

def get_posenc(n, d):
    pos = np.arange(n)[:, None].astype(np.float32)
    i = np.arange(d)[None, :]
    angle_rates = 1.0 / np.power(
        10000.0, (2 * (i // 2)).astype(np.float32) / np.float32(d)
    )
    angles = pos * angle_rates
    pe = np.zeros((n, d), dtype=np.float32)
    pe[:, 0::2] = np.sin(angles[:, 0::2])
    pe[:, 1::2] = np.cos(angles[:, 1::2])
    return pe


_posenc_cache = {}


def posenc_factors(n, d, r=RNK - 2):
    """P ~= Us @ V exactly to ~1e-6 relative: the sinusoidal posenc has a
    sharp numerical-rank cliff at ~470. Randomized SVD (deterministic seed,
    one power iteration) captures the top-r subspace to machine precision."""
    if (n, d, r) in _posenc_cache:
        return _posenc_cache[(n, d, r)]
    Pm = get_posenc(n, d).astype(np.float64)
    rng = np.random.default_rng(12345)
    G = rng.standard_normal((d, r + 32))
    Y = Pm @ G
    Y = Pm @ (Pm.T @ Y)
    Q, _ = np.linalg.qr(Y)
    Bm = Q.T @ Pm
    Ub, s, Vt = np.linalg.svd(Bm, full_matrices=False)
    Us = (Q @ Ub[:, :r]) * s[:r][None, :]
    V = Vt[:r, :]
    out = (Us.astype(np.float32), V.astype(np.float32))
    _posenc_cache[(n, d, r)] = out
    return out


def _host_prep(data, W_exp, b_exp, W_merge):
    """Layout-only host prep (plus the constant posenc factorization);
    returns per-core input maps."""
    B, N, D = data.shape
    NB = N // 2
    NT = N // P
    CT = 2 * NT
    ET = NT

    # rank RNK-2 posenc; rho slots 480/481 (partition base 96 of tile 3)
    # carry the w0*csum rank-2 term: vs col 481 is 1.0 so the on-device
    # inv_k scale turns it into inv_k and Cm[:,:,481] becomes csum.
    Us, V = posenc_factors(N, D)
    pe_slots = np.r_[0:480, 482:RNK]
    vs = np.zeros((N, RNK), dtype=bfloat16)
    vs[:, pe_slots] = V.T.astype(bfloat16)
    vs[:, 481] = 1.0
    us_h = []
    for h in range(2):
        u = np.zeros((RNK, NB), dtype=bfloat16)
        u[pe_slots] = Us[h * NB : (h + 1) * NB, :].T.astype(bfloat16)
        us_h.append(u)

    w1 = np.ascontiguousarray(W_exp[1:].astype(bfloat16))    # [N, D]
    w0h_v = []
    for h in range(2):
        w0h = np.zeros((P, D), dtype=bfloat16)
        w0h[96 + h] = W_exp[0].astype(bfloat16)
        w0h_v.append(w0h)

    wm_s = np.ascontiguousarray(
        W_merge.astype(bfloat16).reshape(CT, P, ET, P).transpose(2, 1, 0, 3)
    )  # [et, p, ct, f]

    bexp_f = np.ascontiguousarray(b_exp.astype(np.float32))

    dataT_b = [np.ascontiguousarray(data[b].T.astype(bfloat16)) for b in range(B)]

    identb = np.eye(P, dtype=np.float32).astype(bfloat16)

    in_maps = []
    for c in range(2 * B):
        b, h = c // 2, c % 2
        nb = slice(h * NB, (h + 1) * NB)
        m = {
            "dataT": dataT_b[b],
            "dataTnb": np.ascontiguousarray(dataT_b[b][:, nb]),
            "vsd": vs,
            "usd": us_h[h],
            "w1d": w1,
            "w0h": w0h_v[h],
            "wm": wm_s,
            "bexp": bexp_f,
            "identb": identb,
        }
        in_maps.append(m)
    return in_maps


_program_cache = {}


def _get_program(N, D, NB, repeat=1, bodies=1, no_dma=False):
    key = (N, D, NB, repeat, bodies, no_dma)
    if key not in _program_cache:
        _program_cache[key] = build_program(
            N, D, NB, repeat=repeat, bodies=bodies, no_dma=no_dma
        )
    return _program_cache[key]


def kernel(data, W_exp, b_exp, W_merge):
    data = np.asarray(data)
    W_exp = np.asarray(W_exp)
    b_exp = np.asarray(b_exp)
    W_merge = np.asarray(W_merge)
    B, N, D = data.shape
    NB = N // 2

    nc = _get_program(N, D, NB)
    in_maps = _host_prep(data, W_exp, b_exp, W_merge)
    core_ids = list(range(2 * B))
    res = run_bass_kernel_spmd(nc, in_maps, core_ids)

    out = np.empty((B, N, D), dtype=np.float32)
    for c in core_ids:
        b, h = c // 2, c % 2
        out[b, h * NB : (h + 1) * NB, :] = res.results[c]["outT"].T
    return out
